# revision 3
# baseline (speedup 1.0000x reference)
"""HHyperGNN (3-layer hypergraph conv) Trainium2 Bass kernel, 8-core SPMD.

Strategy
--------
Per conv layer (D_out = 256, 256, 64pad):
  1. matmul: XW_shard = X_shard @ W, computed from pre-transposed X^T tiles
     (PE systolic, fp32), written node-major.
  2. AllGather XW_shard -> XW_full (every core gets the full gather table).
  3. pass-1 (edge aggregation, scatter_mean): incidences are sharded by the
     OWNER EDGE's core; each core dma_gathers XW rows for its edges'
     incidences and segment-sums them on the PE using host-built 0/1
     selection matrices (contraction over the partition axis = gathered
     rows). Scale rows by homo[e]/cnt[e] -> Xe' table. Layer 0 additionally
     appends a column holding homo[e] (written by scaling a host-provided
     cnt column... see code) so that pass-2's segment sum also produces
     att_sum per node for free.
  4. AllGather Xe' -> full table.
  5. pass-2 (node aggregation): same machinery keyed by owner node; psum
     column D gives att_sum (layer 0; cached reciprocal reused by layers
     1-2), scale by 1/att_sum, add residual XW row, row-L2-normalize,
     relu (layers 0-1), write X' and its PE-transposed tiles for the next
     layer's matmul.

All per-core variation lives in input DATA (indices, selection matrices,
scale columns) -- the instruction stream is identical across cores (SPMD).
Indices for dma_gather are int16, so gather tables are split into chunks of
<= 25088 rows; incidences are grouped by (segment group, chunk) with each
run padded to a static budget so the program is uniform across cores.
"""

import math
import numpy as np

import concourse.bass as bass
import concourse.bacc as bacc
import concourse.mybir as mybir
from concourse import tile
from concourse.bass_utils import run_bass_kernel_spmd

F32 = mybir.dt.float32
I16 = mybir.dt.int16

NC = 8


class Cfg:
    def __init__(self, N, NE, NNZ, nfeat=256, dout=64):
        self.N, self.NE, self.NNZ = N, NE, NNZ
        self.D = nfeat            # hidden dim (= input feat dim)
        self.DOUT = dout          # padded output dim (>= 40, mult of 64)
        self.NPC = N // NC        # nodes per core
        self.EPC = NE // NC       # edges per core
        self.P2G = math.ceil(self.NPC / 128)   # pass-2 groups (node groups)
        self.P1G = math.ceil(self.EPC / 128)   # pass-1 groups (edge groups)
        self.NPC_PAD = self.P2G * 128
        self.EPC_PAD = self.P1G * 128
        self.NTR = NC * self.NPC_PAD           # node table rows (padded)
        self.ETR = NC * self.EPC_PAD           # edge table rows (padded)
        self.NCH1 = self._nchunks(self.NTR)    # chunks of node table
        self.NCH2 = self._nchunks(self.ETR)    # chunks of edge table
        self.CH1 = self.NTR // self.NCH1
        self.CH2 = self.ETR // self.NCH2

    @staticmethod
    def _nchunks(rows):
        for k in (1, 2, 4, 8):
            if rows % k == 0 and rows // k <= 32512:
                return k
        raise ValueError(f"table too large: {rows}")


REAL_CFG = Cfg(100000, 50000, 1000000)


def _wrap_idx(idx):
    """[num] -> [128, num//16] int16 (idx k at partition k%16, col k//16; x8 replicated)."""
    num = idx.shape[0]
    w = idx.reshape(num // 16, 16).T.astype(np.int16)
    return np.tile(w, (8, 1))


def _build_streams(cfg, seg_owner, seg_loc, tbl_row, nchunk, chsz, groups):
    """Build padded incidence streams for one pass.

    seg_owner: owning core per incidence; seg_loc: local segment id on that core;
    tbl_row: padded gather-table row per incidence.
    Returns (SUB, idx [NC,128,calls*SUB*8] int16, S [NC,128,cols] f32 view args)
    """
    g = seg_loc // 128
    slot = seg_loc % 128
    chunk = tbl_row // chsz
    run = (seg_owner * groups + g) * nchunk + chunk
    nruns = NC * groups * nchunk
    counts = np.bincount(run, minlength=nruns)
    sub = max(1, math.ceil(counts.max() / 128))
    rows_per_run = sub * 128
    order = np.argsort(run, kind="stable")
    run_s = run[order]
    start = np.zeros(nruns + 1, np.int64)
    np.cumsum(counts, out=start[1:])
    within = np.arange(run.shape[0], dtype=np.int64) - start[run_s]
    pos = run_s * rows_per_run + within
    rows_total = nruns * rows_per_run
    idx_flat = np.zeros(rows_total, np.int16)
    idx_flat[pos] = (tbl_row[order] - chunk[order] * chsz).astype(np.int16)
    per_core_rows = groups * nchunk * rows_per_run
    core_of = pos // per_core_rows
    lpos = pos - core_of * per_core_rows
    part = lpos % 128
    col = (lpos // 128) * 128 + slot[order]
    S = np.zeros((NC, 128, groups * nchunk * rows_per_run), np.float32)
    S[core_of, part, col] = 1.0
    idx_wrapped = np.stack(
        [_wrap_idx(idx_flat[c * per_core_rows:(c + 1) * per_core_rows]) for c in range(NC)]
    )
    return sub, idx_wrapped, S


def _build_meta(cfg, V, E, homo):
    V = np.asarray(V).astype(np.int64).ravel()
    E = np.asarray(E).astype(np.int64).ravel()
    homo = np.asarray(homo).astype(np.float32).ravel()

    # pass-1: segments = edges, gather table = node-indexed (XW)
    owner_e = E // cfg.EPC
    e_loc = E - owner_e * cfg.EPC
    row_v = (V // cfg.NPC) * cfg.NPC_PAD + (V % cfg.NPC)
    sub1, idx1, S1 = _build_streams(cfg, owner_e, e_loc, row_v, cfg.NCH1, cfg.CH1, cfg.P1G)

    # pass-2: segments = nodes, gather table = edge-indexed (Xe)
    owner_v = V // cfg.NPC
    n_loc = V - owner_v * cfg.NPC
    row_e = (E // cfg.EPC) * cfg.EPC_PAD + (E % cfg.EPC)
    sub2, idx2, S2 = _build_streams(cfg, owner_v, n_loc, row_e, cfg.NCH2, cfg.CH2, cfg.P2G)

    # per-edge columns, wrapped [NC, 128, P1G]: slot (g,p) <-> local edge g*128+p
    cnt = np.bincount(E, minlength=cfg.NE).astype(np.float64)
    invcnt = (1.0 / np.maximum(cnt, 1.0)).astype(np.float32)
    invcnt_col = np.zeros((NC, 128, cfg.P1G), np.float32)
    homo_col = np.zeros((NC, 128, cfg.P1G), np.float32)
    eids = np.arange(cfg.EPC)
    gg, pp = eids // 128, eids % 128
    for c in range(NC):
        invcnt_col[c, pp, gg] = invcnt[c * cfg.EPC + eids]
        homo_col[c, pp, gg] = homo[c * cfg.EPC + eids]
    return sub1, idx1, S1, sub2, idx2, S2, invcnt_col, homo_col


def _build_xt0(cfg, X):
    """Host pre-transpose of the layer-0 input: per core [128, P2G*2*128],
    [p, (t*2+b)*128 + m] = X[core*NPC + t*128 + m, b*128 + p] (0 past NPC)."""
    D = cfg.D
    out = np.zeros((NC, 128, cfg.P2G * (D // 128) * 128), np.float32)
    for c in range(NC):
        xs = X[c * cfg.NPC:(c + 1) * cfg.NPC]  # [NPC, D]
        xp = np.zeros((cfg.NPC_PAD, D), np.float32)
        xp[:cfg.NPC] = xs
        # [t, m, b, p] -> [p, t, b, m] so cols are ((t*NB+b)*128 + m)
        t4 = xp.reshape(cfg.P2G, 128, D // 128, 128).transpose(3, 0, 2, 1)
        out[c] = np.ascontiguousarray(t4).reshape(128, -1)
    return out


def _gen_nc(cfg, sub1, sub2):
    D = cfg.D
    DO3 = cfg.DOUT
    nc = bacc.Bacc("TRN2", target_bir_lowering=False, debug=False, num_devices=NC)

    # ---------------- I/O ----------------
    xt0 = nc.dram_tensor("xt0", [128, cfg.P2G * (D // 128) * 128], F32, kind="ExternalInput")
    w1 = nc.dram_tensor("w1", [D, D], F32, kind="ExternalInput")
    w2 = nc.dram_tensor("w2", [D, D], F32, kind="ExternalInput")
    w3 = nc.dram_tensor("w3", [D, DO3], F32, kind="ExternalInput")
    idx1_d = nc.dram_tensor("idx1", [128, cfg.P1G * cfg.NCH1 * sub1 * 8], I16, kind="ExternalInput")
    idx2_d = nc.dram_tensor("idx2", [128, cfg.P2G * cfg.NCH2 * sub2 * 8], I16, kind="ExternalInput")
    s1_d = nc.dram_tensor("s1", [128, cfg.P1G * cfg.NCH1 * sub1 * 128], F32, kind="ExternalInput")
    s2_d = nc.dram_tensor("s2", [128, cfg.P2G * cfg.NCH2 * sub2 * 128], F32, kind="ExternalInput")
    invcnt_d = nc.dram_tensor("invcnt", [128, cfg.P1G], F32, kind="ExternalInput")
    homoc_d = nc.dram_tensor("homoc", [128, cfg.P1G], F32, kind="ExternalInput")
    ident_d = nc.dram_tensor("ident", [128, 128], F32, kind="ExternalInput")
    z_out = nc.dram_tensor("Z", [cfg.NPC_PAD, D], F32, kind="ExternalOutput")
    x_out = nc.dram_tensor("XO", [cfg.NPC_PAD, DO3], F32, kind="ExternalOutput")

    DOUT_L = [D, D, DO3]
    EL2_L = [D + 64, D, DO3]     # xe table row width per layer (layer0 has att col)
    RHS_L = [D + 1, D, DO3]      # matmul moving width in pass-2

    with tile.TileContext(nc) as tc:
        with (
            tc.tile_pool(name="dram", bufs=1, space="DRAM") as dram,
            tc.tile_pool(name="const", bufs=1) as constp,
            tc.tile_pool(name="idxp", bufs=1) as idxp,
            tc.tile_pool(name="gb", bufs=3) as gbp,
            tc.tile_pool(name="sb", bufs=3) as sbp,
            tc.tile_pool(name="xt", bufs=3) as xtp,
            tc.tile_pool(name="st", bufs=3) as stp,
            tc.tile_pool(name="rw", bufs=2) as rwp,
            tc.tile_pool(name="sc", bufs=2) as scp,
            tc.tile_pool(name="col", bufs=4) as colp,
            tc.tile_pool(name="pmm", bufs=2, space="PSUM") as pmm,
            tc.tile_pool(name="pseg", bufs=2, space="PSUM") as pseg,
            tc.tile_pool(name="ptr", bufs=2, space="PSUM") as ptr,
        ):
            # ---- persistent DRAM intermediates ----
            xw_shard = [dram.tile([cfg.NPC_PAD, DOUT_L[l]], F32, tag=f"xws{l}", name=f"xws{l}") for l in range(3)]
            xw_full = [dram.tile([cfg.NTR, DOUT_L[l]], F32, addr_space="Shared", tag=f"xwf{l}", name=f"xwf{l}")
                       for l in range(3)]
            xe_shard = [dram.tile([cfg.EPC_PAD, EL2_L[l]], F32, tag=f"xes{l}", name=f"xes{l}") for l in range(3)]
            xe_full = [dram.tile([cfg.ETR, EL2_L[l]], F32, addr_space="Shared", tag=f"xef{l}", name=f"xef{l}")
                       for l in range(3)]
            x_loc = [dram.tile([cfg.NPC_PAD, D], F32, tag=f"xl{l}", name=f"xl{l}") for l in range(2)]
            xt_loc = [dram.tile([128, cfg.P2G * (D // 128) * 128], F32, tag=f"xtl{l}", name=f"xtl{l}")
                      for l in range(2)]

            # ---- persistent SBUF constants ----
            ident = constp.tile([128, 128], F32)
            nc.sync.dma_start(out=ident[:, :], in_=ident_d[:, :])
            idx1_t = idxp.tile([128, cfg.P1G * cfg.NCH1 * sub1 * 8], I16)
            nc.sync.dma_start(out=idx1_t[:, :], in_=idx1_d[:, :])
            idx2_t = idxp.tile([128, cfg.P2G * cfg.NCH2 * sub2 * 8], I16)
            nc.sync.dma_start(out=idx2_t[:, :], in_=idx2_d[:, :])
            invcnt_t = constp.tile([128, cfg.P1G], F32)
            nc.sync.dma_start(out=invcnt_t[:, :], in_=invcnt_d[:, :])
            homoc_t = constp.tile([128, cfg.P1G], F32)
            nc.sync.dma_start(out=homoc_t[:, :], in_=homoc_d[:, :])
            s1col = constp.tile([128, cfg.P1G], F32)
            nc.vector.tensor_mul(s1col[:, :], homoc_t[:, :], invcnt_t[:, :])
            inv_att = constp.tile([128, cfg.P2G], F32)

            wt = [w1, w2, w3]
            w_sb = []
            for l in range(3):
                wtl = constp.tile([128, (D // 128) * DOUT_L[l]], F32, tag=f"w{l}", name=f"w{l}")
                for b in range(D // 128):
                    nc.sync.dma_start(
                        out=wtl[:, b * DOUT_L[l]:(b + 1) * DOUT_L[l]],
                        in_=wt[l][b * 128:(b + 1) * 128, :])
                w_sb.append(wtl)

            NB = D // 128  # feature blocks (2)

            for l in range(3):
                DO = DOUT_L[l]
                EL2 = EL2_L[l]
                RHS = RHS_L[l]

                # ---- A: matmul XW_shard = X_shard @ W ----
                xt_src = xt0 if l == 0 else xt_loc[l - 1]
                for t in range(cfg.P2G):
                    xtb = xtp.tile([128, NB * 128], F32, tag="xtb")
                    nc.sync.dma_start(out=xtb[:, :], in_=xt_src[:, t * NB * 128:(t + 1) * NB * 128])
                    pm = pmm.tile([128, DO], F32, tag="pmm")
                    for b in range(NB):
                        nc.tensor.matmul(pm[:, :], xtb[:, b * 128:(b + 1) * 128],
                                         w_sb[l][:, b * DO:(b + 1) * DO],
                                         start=(b == 0), stop=(b == NB - 1))
                    stm = stp.tile([128, DO], F32, tag="stm")
                    nc.vector.tensor_copy(stm[:, :], pm[:, :])
                    nc.sync.dma_start(out=xw_shard[l][t * 128:(t + 1) * 128, :], in_=stm[:, :])

                # ---- B: AllGather XW ----
                nc.gpsimd.collective_compute(
                    "AllGather", mybir.AluOpType.bypass,
                    replica_groups=[list(range(NC))],
                    ins=[xw_shard[l].opt()], outs=[xw_full[l].opt()])

                # ---- C: pass-1 edge aggregation ----
                for g in range(cfg.P1G):
                    ps = pseg.tile([128, DO], F32, tag="ps1")
                    si = 0
                    tot = cfg.NCH1 * sub1
                    for k in range(cfg.NCH1):
                        call = g * cfg.NCH1 + k
                        gb = gbp.tile([128, sub1, DO], F32, tag="gb")
                        nc.gpsimd.dma_gather(
                            out_ap=gb[:, :, :],
                            in_ap=xw_full[l][k * cfg.CH1:(k + 1) * cfg.CH1, :],
                            idxs_ap=idx1_t[:, call * sub1 * 8:(call + 1) * sub1 * 8],
                            num_idxs=sub1 * 128, num_idxs_reg=sub1 * 128,
                            elem_size=DO, single_packet=False)
                        st_ = sbp.tile([128, sub1 * 128], F32, tag="S")
                        nc.sync.dma_start(
                            out=st_[:, :],
                            in_=s1_d[:, call * sub1 * 128:(call + 1) * sub1 * 128])
                        for j in range(sub1):
                            nc.tensor.matmul(ps[:, :], st_[:, j * 128:(j + 1) * 128],
                                             gb[:, j, 0:DO],
                                             start=(si == 0), stop=(si == tot - 1))
                            si += 1
                    stg = stp.tile([128, EL2_L[0]], F32, tag="stg")
                    nc.vector.tensor_scalar_mul(stg[:, 0:DO], ps[:, :], s1col[:, g:g + 1])
                    if l == 0:
                        nc.vector.tensor_copy(stg[:, DO:DO + 1], homoc_t[:, g:g + 1])
                        nc.sync.dma_start(out=xe_shard[l][g * 128:(g + 1) * 128, 0:DO + 1],
                                          in_=stg[:, 0:DO + 1])
                    else:
                        nc.sync.dma_start(out=xe_shard[l][g * 128:(g + 1) * 128, :],
                                          in_=stg[:, 0:DO])

                # ---- D: AllGather Xe ----
                nc.gpsimd.collective_compute(
                    "AllGather", mybir.AluOpType.bypass,
                    replica_groups=[list(range(NC))],
                    ins=[xe_shard[l].opt()], outs=[xe_full[l].opt()])

                # ---- E: pass-2 node aggregation + finish ----
                for g in range(cfg.P2G):
                    ps = pseg.tile([128, RHS_L[0]], F32, tag="ps2")
                    si = 0
                    tot = cfg.NCH2 * sub2
                    for k in range(cfg.NCH2):
                        call = g * cfg.NCH2 + k
                        gb = gbp.tile([128, sub2, EL2], F32, tag="gb")
                        nc.gpsimd.dma_gather(
                            out_ap=gb[:, :, :],
                            in_ap=xe_full[l][k * cfg.CH2:(k + 1) * cfg.CH2, :],
                            idxs_ap=idx2_t[:, call * sub2 * 8:(call + 1) * sub2 * 8],
                            num_idxs=sub2 * 128, num_idxs_reg=sub2 * 128,
                            elem_size=EL2, single_packet=False)
                        st_ = sbp.tile([128, sub2 * 128], F32, tag="S")
                        nc.sync.dma_start(
                            out=st_[:, :],
                            in_=s2_d[:, call * sub2 * 128:(call + 1) * sub2 * 128])
                        for j in range(sub2):
                            nc.tensor.matmul(ps[:, 0:RHS], st_[:, j * 128:(j + 1) * 128],
                                             gb[:, j, 0:RHS],
                                             start=(si == 0), stop=(si == tot - 1))
                            si += 1
                    if l == 0:
                        att = colp.tile([128, 1], F32, tag="att")
                        nc.vector.tensor_scalar_max(att[:, :], ps[:, D:D + 1], 1e-20)
                        nc.vector.reciprocal(inv_att[:, g:g + 1], att[:, :])
                    stg = stp.tile([128, D], F32, tag="stx")
                    nc.vector.tensor_scalar_mul(stg[:, 0:DO], ps[:, 0:DO], inv_att[:, g:g + 1])
                    rw = rwp.tile([128, DO], F32, tag="rw")
                    nc.sync.dma_start(out=rw[:, :], in_=xw_shard[l][g * 128:(g + 1) * 128, :])
                    nc.vector.tensor_add(stg[:, 0:DO], stg[:, 0:DO], rw[:, :])
                    # row L2 normalize
                    sq = scp.tile([128, DO], F32, tag="sq")
                    ssq = colp.tile([128, 1], F32, tag="ssq")
                    nc.scalar.activation(sq[:, :], stg[:, 0:DO],
                                         mybir.ActivationFunctionType.Square,
                                         accum_out=ssq[:, :])
                    nc.vector.tensor_scalar_max(ssq[:, :], ssq[:, :], 1e-30)
                    nrm = colp.tile([128, 1], F32, tag="nrm")
                    nc.scalar.sqrt(nrm[:, :], ssq[:, :])
                    rn = colp.tile([128, 1], F32, tag="rn")
                    nc.vector.reciprocal(rn[:, :], nrm[:, :])
                    nc.vector.tensor_scalar_mul(stg[:, 0:DO], stg[:, 0:DO], rn[:, :])
                    if l < 2:
                        nc.vector.tensor_scalar_max(stg[:, 0:DO], stg[:, 0:DO], 0.0)
                        dst = x_loc[0] if l == 0 else z_out
                        nc.sync.dma_start(out=dst[g * 128:(g + 1) * 128, :], in_=stg[:, 0:DO])
                        # transposed tiles for next matmul
                        for b in range(NB):
                            pt = ptr.tile([128, 128], F32, tag="pt")
                            nc.tensor.transpose(pt[:, :], stg[:, b * 128:(b + 1) * 128],
                                                ident[:, :])
                            xts = xtp.tile([128, 128], F32, tag="xts")
                            nc.vector.tensor_copy(xts[:, :], pt[:, :])
                            nc.sync.dma_start(
                                out=xt_loc[l][:, (g * NB + b) * 128:(g * NB + b + 1) * 128],
                                in_=xts[:, :])
                    else:
                        nc.sync.dma_start(out=x_out[g * 128:(g + 1) * 128, :], in_=stg[:, 0:DO])

    nc.compile()
    return nc


def _prepare(cfg, X, V, E, homo, W1, W2, Wout):
    X = np.ascontiguousarray(np.asarray(X, dtype=np.float32))
    W1 = np.asarray(W1, dtype=np.float32)
    W2 = np.asarray(W2, dtype=np.float32)
    Wout = np.asarray(Wout, dtype=np.float32)
    w3 = np.zeros((cfg.D, cfg.DOUT), np.float32)
    w3[:, :Wout.shape[1]] = Wout
    sub1, idx1, S1, sub2, idx2, S2, invcnt_col, homo_col = _build_meta(cfg, V, E, homo)
    xt0 = _build_xt0(cfg, X)
    ident = np.eye(128, dtype=np.float32)
    in_maps = []
    for c in range(NC):
        in_maps.append({
            "xt0": xt0[c], "w1": W1, "w2": W2, "w3": w3,
            "idx1": np.ascontiguousarray(idx1[c]),
            "idx2": np.ascontiguousarray(idx2[c]),
            "s1": np.ascontiguousarray(S1[c]),
            "s2": np.ascontiguousarray(S2[c]),
            "invcnt": np.ascontiguousarray(invcnt_col[c]),
            "homoc": np.ascontiguousarray(homo_col[c]),
            "ident": ident,
        })
    return sub1, sub2, in_maps


def _run(cfg, X, V, E, homo, W1, W2, Wout, nouts=40):
    sub1, sub2, in_maps = _prepare(cfg, X, V, E, homo, W1, W2, Wout)
    nc = _gen_nc(cfg, sub1, sub2)
    res = run_bass_kernel_spmd(nc, in_maps, core_ids=list(range(NC)))
    Z = np.concatenate([res.results[c]["Z"][:cfg.NPC] for c in range(NC)], axis=0)
    XO = np.concatenate([res.results[c]["XO"][:cfg.NPC] for c in range(NC)], axis=0)
    return Z, XO[:, :nouts]


def kernel(X, V, E, homo, W1, W2, Wout):
    return _run(REAL_CFG, X, V, E, homo, W1, W2, Wout, nouts=np.asarray(Wout).shape[1])
